# revision 6
# baseline (speedup 1.0000x reference)
"""OffsetMSE loss kernel for 8x Trainium2 NeuronCores.

Math: reference computes, for shifts s in 1..95,
    loss(s) = sum_b sum_{i<L-s} (p[b,i+s] - q[b,i])^2 / (B*(L-s))
and returns min_s loss(s).

Decomposition:
    loss(s)*B*(L-s) = A(s) - 2*X(s) + C(s)
      A(s) = SP - prefix_p(s)        SP = sum p^2,  prefix_p(s) = sum_b sum_{j<s} p^2
      C(s) = SQ - suffix_q(s)        SQ = sum q^2,  suffix_q(s) = sum_b sum_{i>=L-s} q^2
      X(s) = sum_b sum_j p[b,j+s]*q[b,j]   (p zero-padded past L)

The heavy terms (X for all 95 lags, SP, SQ) are computed on-device:
batch dim is sharded 2 sequences/core across 8 cores. Each core tiles its
data into "superblocks" of 128 rows x 2048, casts to fp16 (ACT casts p,
DVE casts q), and for each 128-column chunk runs PE matmuls with the q
chunk stationary:
  X:  moving = p window (128x224)  -> PSUM OUT[k,t]  = sum_u q[u+k]p[u+t]
  QQ: moving = q chunk  (128x128)  -> PSUM QQ[k,t']  = sum_u q[u+k]q[u+t']
  PP: stationary/moving = p chunk  -> PSUM PP[k,t']  = sum_u p[u+k]p[u+t']
accumulated over all chunks. Diagonal sums give X(s) = sum_k OUT[k,k+s],
SQ = tr(QQ), SP = tr(PP). Host combines partials (tiny, O(128*480)).
"""

import os
import sys

os.environ.setdefault("MYCRO_LOCAL_CACHE", "1")
if "/opt/trn_rl_repo" not in sys.path:
    sys.path.insert(0, "/opt/trn_rl_repo")

import numpy as np

_L = 1048576
_B = 16
_NCORES = 8
_BPC = _B // _NCORES  # sequences per core = 2
_P = 128
_W = 2048             # row width within a superblock
_SB = _W * _P         # superblock elements = 262144
_NSB = _L // _SB      # superblocks per sequence = 4
_NCHUNK = _W // 128   # 16
_S = 96               # max shift (exclusive); shifts used are 1..95
_NW = 128 + _S        # moving-operand window = 224
_PEXT = _W + _S       # p tile free extent = 2144 (rows overlap by 96)

TRACE = False
LAST_RESULTS = None

_NC_CACHE = None


def _build(rep=1):
    """rep>1 repeats the whole pass inside one NEFF (benchmarking only:
    output values then accumulate rep times)."""
    from concourse import bacc, mybir
    import concourse.bass as bass
    from concourse.tile import TileContext

    f32 = mybir.dt.float32
    f16 = mybir.dt.float16

    nc = bacc.Bacc(
        "TRN2", target_bir_lowering=False, debug=False, enable_asserts=False
    )
    p_in = nc.dram_tensor("p", [_BPC, _L], f32, kind="ExternalInput")
    q_in = nc.dram_tensor("q", [_BPC, _L], f32, kind="ExternalInput")
    # xout columns: [0:224) X correlation matrix, [224:352) QQ, [352:480) PP
    _XC = _NW + 128 + 128
    xout = nc.dram_tensor("xout", [_P, _XC], f32, kind="ExternalOutput")

    nsb_total = _BPC * _NSB  # 8
    n_mms = rep * nsb_total * _NCHUNK  # 128 per PSUM target (per rep)

    with TileContext(nc) as tc:
        with (
            tc.tile_pool(name="io", bufs=3) as io_pool,
            tc.tile_pool(name="c16", bufs=3) as c16_pool,
            tc.tile_pool(name="scr", bufs=1) as scr_pool,
            tc.tile_pool(name="psum", bufs=1, space="PSUM") as psum_pool,
        ):
            psum_x = psum_pool.tile([_P, _NW], f32, tag="px")
            psum_qq = psum_pool.tile([_P, 128], f32, tag="pq")
            psum_pp = psum_pool.tile([_P, 128], f32, tag="pp")

            mm = 0
            for b in range(rep * _BPC):
                b = b % _BPC
                for sb in range(_NSB):
                    off = b * _L + sb * _SB
                    p_tile = io_pool.tile([_P, _PEXT], f32, tag="p")
                    q_tile = io_pool.tile([_P, _W], f32, tag="q")
                    nc.sync.dma_start(
                        out=q_tile[:, :],
                        in_=bass.AP(q_in, off, [[_W, _P], [1, _W]]),
                    )
                    if sb < _NSB - 1:
                        # overlapping rows: row m covers p[off+W*m : off+W*m+PEXT)
                        nc.sync.dma_start(
                            out=p_tile[:, :],
                            in_=bass.AP(p_in, off, [[_W, _P], [1, _PEXT]]),
                        )
                    else:
                        # last superblock of the sequence: row 127's tail would
                        # run past the sequence end -> zero it (p zero-padding).
                        # memset whole tail block first (engines need aligned
                        # start partition); rows 0..126 are then overwritten
                        # with real data.
                        nc.vector.memset(p_tile[:, _W:_PEXT], 0.0)
                        nc.sync.dma_start(
                            out=p_tile[0 : _P - 1, :],
                            in_=bass.AP(p_in, off, [[_W, _P - 1], [1, _PEXT]]),
                        )
                        nc.sync.dma_start(
                            out=p_tile[_P - 1 : _P, 0:_W],
                            in_=bass.AP(p_in, off + _W * (_P - 1), [[_W, 1], [1, _W]]),
                        )

                    p16 = c16_pool.tile([_P, _PEXT], f16, tag="p16")
                    q16 = c16_pool.tile([_P, _W], f16, tag="q16")
                    nc.scalar.copy(p16[:, :], p_tile[:, :])
                    nc.vector.tensor_copy(q16[:, :], q_tile[:, :])

                    first = mm == 0
                    last = mm == n_mms - 1
                    for c in range(_NCHUNK):
                        first = mm == 0
                        last = mm == n_mms - 1
                        qc = q16[:, 128 * c : 128 * c + 128]
                        pc = p16[:, 128 * c : 128 * c + 128]
                        nc.tensor.matmul(
                            psum_x[:, :],
                            qc,
                            p16[:, 128 * c : 128 * c + _NW],
                            start=first,
                            stop=last,
                        )
                        nc.tensor.matmul(
                            psum_qq[:, :], qc, qc, start=first, stop=last
                        )
                        nc.tensor.matmul(
                            psum_pp[:, :], pc, pc, start=first, stop=last
                        )
                        mm += 1

            out_sb = scr_pool.tile([_P, _XC], f32, tag="ox")
            nc.vector.tensor_copy(out_sb[:, 0:_NW], psum_x[:, :])
            nc.vector.tensor_copy(out_sb[:, _NW : _NW + 128], psum_qq[:, :])
            nc.vector.tensor_copy(out_sb[:, _NW + 128 : _XC], psum_pp[:, :])
            nc.sync.dma_start(out=xout[:, :], in_=out_sb[:, :])

    nc.compile()
    return nc


def _get_nc():
    global _NC_CACHE
    if _NC_CACHE is None:
        _NC_CACHE = _build()
    return _NC_CACHE


def _run_device(p, q):
    """p, q: (16, L) float32. Returns xout (128 x 480 f64) summed over cores."""
    global LAST_RESULTS
    from concourse import bass_utils

    nc = _get_nc()
    in_maps = [
        {
            "p": np.ascontiguousarray(p[_BPC * c : _BPC * (c + 1)]),
            "q": np.ascontiguousarray(q[_BPC * c : _BPC * (c + 1)]),
        }
        for c in range(_NCORES)
    ]
    if os.environ.get("BASS_BACKEND", "hw") == "sim":
        from concourse.bass_interp import CoreSim

        res_list = []
        for c in range(_NCORES):
            sim = CoreSim(nc)
            sim.tensor("p")[:] = in_maps[c]["p"]
            sim.tensor("q")[:] = in_maps[c]["q"]
            sim.simulate()
            res_list.append({"xout": np.array(sim.tensor("xout"))})
    else:
        res = bass_utils.run_bass_kernel_spmd(
            nc, in_maps, core_ids=list(range(_NCORES)), trace=TRACE
        )
        LAST_RESULTS = res
        res_list = res.results

    OUT = np.zeros((_P, _NW + 256), dtype=np.float64)
    for r in res_list:
        OUT += r["xout"].astype(np.float64)
    return OUT


def kernel(predict, target):
    p = np.ascontiguousarray(predict.reshape(_B, _L)).astype(np.float32, copy=False)
    q = np.ascontiguousarray(target.reshape(_B, _L)).astype(np.float32, copy=False)

    OUT = _run_device(p, q)

    s = np.arange(1, _S)  # shifts 1..95
    k = np.arange(_P)
    X = OUT[:, 0:_NW][k[:, None], k[:, None] + s[None, :]].sum(axis=0)  # (95,)
    SQ = np.trace(OUT[:, _NW : _NW + 128])
    SP = np.trace(OUT[:, _NW + 128 : _NW + 256])

    # tiny edge terms from the raw inputs (O(B*S) work)
    phead = (p[:, : _S - 1].astype(np.float64) ** 2).sum(axis=0)  # j = 0..94
    prefix = np.concatenate([[0.0], np.cumsum(phead)])  # prefix[s] = sum_{j<s}
    qtail = (q[:, _L - (_S - 1) :].astype(np.float64) ** 2).sum(axis=0)
    suffix = np.concatenate([[0.0], np.cumsum(qtail[::-1])])  # suffix[s] = last s

    losses = (SP - prefix[s] + SQ - suffix[s] - 2.0 * X) / (
        float(_B) * (_L - s).astype(np.float64)
    )
    return np.asarray(losses.min(), dtype=np.float32)


# revision 16
# speedup vs baseline: 9.7726x; 9.7726x over previous
"""OffsetMSE loss kernel for 8x Trainium2 NeuronCores.

Math: reference computes, for shifts s in 1..95,
    loss(s) = sum_b sum_{i<L-s} (p[b,i+s] - q[b,i])^2 / (B*(L-s))
and returns min_s loss(s).

Decomposition:
    loss(s)*B*(L-s) = A(s) - 2*X(s) + C(s)
      A(s) = SP - prefix_p(s)        SP = sum p^2,  prefix_p(s) = sum_b sum_{j<s} p^2
      C(s) = SQ - suffix_q(s)        SQ = sum q^2,  suffix_q(s) = sum_b sum_{i>=L-s} q^2
      X(s) = sum_b sum_j p[b,j+s]*q[b,j]   (p zero-padded past L)

The heavy terms (X for all 95 lags, SP, SQ) are computed on-device:
batch dim is sharded 2 sequences/core across 8 cores. Each core tiles its
data into "superblocks" of 128 rows x 2048, casts to fp16 (ACT casts p,
DVE casts q), and for each 128-column chunk runs PE matmuls with the q
chunk stationary:
  X:  moving = p window (128x224)  -> PSUM OUT[k,t]  = sum_u q[u+k]p[u+t]
  QQ: moving = q chunk  (128x128)  -> PSUM QQ[k,t']  = sum_u q[u+k]q[u+t']
  PP: stationary/moving = p chunk  -> PSUM PP[k,t']  = sum_u p[u+k]p[u+t']
accumulated over all chunks. Diagonal sums give X(s) = sum_k OUT[k,k+s],
SQ = tr(QQ), SP = tr(PP). Host combines partials (tiny, O(128*480)).
"""

import os
import sys

os.environ.setdefault("MYCRO_LOCAL_CACHE", "1")
if "/opt/trn_rl_repo" not in sys.path:
    sys.path.insert(0, "/opt/trn_rl_repo")

import numpy as np

_L = 1048576
_B = 16
_NCORES = 8
_BPC = _B // _NCORES  # sequences per core = 2
_P = 128
_W = 2048             # row width within a superblock
_SB = _W * _P         # superblock elements = 262144
_NSB = _L // _SB      # superblocks per sequence = 4
_NCHUNK = _W // 128   # 16
_S = 96               # max shift (exclusive); shifts used are 1..95
_NW = 128 + _S        # moving-operand window = 224
_PEXT = _W + _S       # p tile free extent = 2144 (rows overlap by 96)

TRACE = False
LAST_RESULTS = None

_NC_CACHE = None


def _build(rep=1, io_bufs=None, c16_bufs=None, loop_n=None):
    """rep>1 repeats the whole pass inside one NEFF (benchmarking only:
    output values then accumulate rep times). loop_n wraps the pass in a
    hardware For_i loop of loop_n iterations (benchmarking only)."""
    from concourse import bacc, mybir
    import concourse.bass as bass
    from concourse.tile import TileContext
    from contextlib import nullcontext

    if io_bufs is None:
        io_bufs = int(os.environ.get("K_IO_BUFS", "4"))
    if c16_bufs is None:
        c16_bufs = int(os.environ.get("K_C16_BUFS", "4"))
    split = int(os.environ.get("K_SPLIT", "2"))  # column-split units/superblock

    f32 = mybir.dt.float32
    f16 = mybir.dt.float16

    nc = bacc.Bacc(
        "TRN2", target_bir_lowering=False, debug=False, enable_asserts=False
    )
    p_in = nc.dram_tensor("p", [_BPC, _L], f32, kind="ExternalInput")
    q_in = nc.dram_tensor("q", [_BPC, _L], f32, kind="ExternalInput")
    # xout columns: [0:224) X correlation matrix, [224:352) QQ, [352:480) PP
    _XC = _NW + 128 + 128
    xout = nc.dram_tensor("xout", [_P, _XC], f32, kind="ExternalOutput")

    nsb_total = _BPC * _NSB  # 8
    n_mms = rep * nsb_total * _NCHUNK  # 128 per PSUM target (per rep)

    with TileContext(nc) as tc:
        with (
            tc.tile_pool(name="io", bufs=io_bufs) as io_pool,
            tc.tile_pool(name="c16", bufs=c16_bufs) as c16_pool,
            tc.tile_pool(name="scr", bufs=1) as scr_pool,
            tc.tile_pool(name="psum", bufs=1, space="PSUM") as psum_pool,
        ):
            psum_x = psum_pool.tile([_P, _NW], f32, tag="px")
            psum_qq = psum_pool.tile([_P, 128], f32, tag="pq")
            psum_pp = psum_pool.tile([_P, 128], f32, tag="pp")

            loop_ctx = tc.For_i(0, loop_n, 1) if loop_n else nullcontext()
            with loop_ctx:
                # Each superblock is processed in column units so casts and
                # matmuls pipeline behind the DMAs at sub-superblock
                # granularity (shrinks the serial tail after the last load).
                # `pattern` lists chunks per unit (sums to _NCHUNK); a small
                # final unit minimizes work after the very last input DMA.
                pat_env = os.environ.get("K_PATTERN", "")
                if pat_env:
                    pattern = [int(x) for x in pat_env.split(",")]
                else:
                    pattern = [_NCHUNK // split] * split
                assert sum(pattern) == _NCHUNK
                bounds = [0]
                for n in pattern:
                    bounds.append(bounds[-1] + n)
                mm = 0
                for b in range(rep * _BPC):
                    b = b % _BPC
                    for sb in range(_NSB):
                        off = b * _L + sb * _SB
                        last_sb = sb == _NSB - 1
                        p_tile = io_pool.tile([_P, _PEXT], f32, tag="p")
                        q_tile = io_pool.tile([_P, _W], f32, tag="q")
                        p16 = c16_pool.tile([_P, _PEXT], f16, tag="p16")
                        q16 = c16_pool.tile([_P, _W], f16, tag="q16")
                        if last_sb:
                            # row 127's tail would run past the sequence end ->
                            # zero it (p zero-padding). memset whole tail-col
                            # block (engines need aligned start partition);
                            # rows 0..126 are overwritten with real data below.
                            nc.vector.memset(p_tile[:, _W:_PEXT], 0.0)
                        for u in range(len(pattern)):
                            qlo, qhi = 128 * bounds[u], 128 * bounds[u + 1]
                            # p columns [plo, phi) for this unit (units chain
                            # non-overlapping; unit 0 includes the 96-col head)
                            plo = qlo + (_S if u > 0 else 0)
                            phi = qhi + _S
                            nc.sync.dma_start(
                                out=q_tile[:, qlo:qhi],
                                in_=bass.AP(
                                    q_in, off + qlo, [[_W, _P], [1, qhi - qlo]]
                                ),
                            )
                            if not (last_sb and u == len(pattern) - 1):
                                nc.sync.dma_start(
                                    out=p_tile[:, plo:phi],
                                    in_=bass.AP(
                                        p_in, off + plo, [[_W, _P], [1, phi - plo]]
                                    ),
                                )
                            else:
                                nc.sync.dma_start(
                                    out=p_tile[0 : _P - 1, plo:phi],
                                    in_=bass.AP(
                                        p_in,
                                        off + plo,
                                        [[_W, _P - 1], [1, phi - plo]],
                                    ),
                                )
                                nc.sync.dma_start(
                                    out=p_tile[_P - 1 : _P, plo:_W],
                                    in_=bass.AP(
                                        p_in,
                                        off + _W * (_P - 1) + plo,
                                        [[_W, 1], [1, _W - plo]],
                                    ),
                                )
                            nc.scalar.copy(p16[:, plo:phi], p_tile[:, plo:phi])
                            nc.vector.tensor_copy(
                                q16[:, qlo:qhi], q_tile[:, qlo:qhi]
                            )
                            for c in range(bounds[u], bounds[u + 1]):
                                first = mm == 0
                                last = mm == n_mms - 1
                                qc = q16[:, 128 * c : 128 * c + 128]
                                pc = p16[:, 128 * c : 128 * c + 128]
                                nc.tensor.matmul(
                                    psum_x[:, :],
                                    qc,
                                    p16[:, 128 * c : 128 * c + _NW],
                                    start=first,
                                    stop=last,
                                )
                                nc.tensor.matmul(
                                    psum_qq[:, :], qc, qc, start=first, stop=last
                                )
                                nc.tensor.matmul(
                                    psum_pp[:, :], pc, pc, start=first, stop=last
                                )
                                mm += 1

            out_sb = scr_pool.tile([_P, _XC], f32, tag="ox")
            nc.vector.tensor_copy(out_sb[:, 0:_NW], psum_x[:, :])
            nc.scalar.copy(out_sb[:, _NW : _NW + 128], psum_qq[:, :])
            nc.vector.tensor_copy(out_sb[:, _NW + 128 : _XC], psum_pp[:, :])
            nc.sync.dma_start(out=xout[:, :], in_=out_sb[:, :])

    nc.compile()
    return nc


def _get_nc():
    global _NC_CACHE
    if _NC_CACHE is None:
        _NC_CACHE = _build()
    return _NC_CACHE


def _run_device(p, q):
    """p, q: (16, L) float32. Returns xout (128 x 480 f64) summed over cores."""
    global LAST_RESULTS
    from concourse import bass_utils

    nc = _get_nc()
    in_maps = [
        {
            "p": np.ascontiguousarray(p[_BPC * c : _BPC * (c + 1)]),
            "q": np.ascontiguousarray(q[_BPC * c : _BPC * (c + 1)]),
        }
        for c in range(_NCORES)
    ]
    if os.environ.get("BASS_BACKEND", "hw") == "sim":
        from concourse.bass_interp import CoreSim

        res_list = []
        for c in range(_NCORES):
            sim = CoreSim(nc)
            sim.tensor("p")[:] = in_maps[c]["p"]
            sim.tensor("q")[:] = in_maps[c]["q"]
            sim.simulate()
            res_list.append({"xout": np.array(sim.tensor("xout"))})
    else:
        res = bass_utils.run_bass_kernel_spmd(
            nc, in_maps, core_ids=list(range(_NCORES)), trace=TRACE
        )
        LAST_RESULTS = res
        res_list = res.results

    OUT = np.zeros((_P, _NW + 256), dtype=np.float64)
    for r in res_list:
        OUT += r["xout"].astype(np.float64)
    return OUT


def kernel(predict, target):
    p = np.ascontiguousarray(predict.reshape(_B, _L)).astype(np.float32, copy=False)
    q = np.ascontiguousarray(target.reshape(_B, _L)).astype(np.float32, copy=False)

    OUT = _run_device(p, q)

    s = np.arange(1, _S)  # shifts 1..95
    k = np.arange(_P)
    X = OUT[:, 0:_NW][k[:, None], k[:, None] + s[None, :]].sum(axis=0)  # (95,)
    SQ = np.trace(OUT[:, _NW : _NW + 128])
    SP = np.trace(OUT[:, _NW + 128 : _NW + 256])

    # tiny edge terms from the raw inputs (O(B*S) work)
    phead = (p[:, : _S - 1].astype(np.float64) ** 2).sum(axis=0)  # j = 0..94
    prefix = np.concatenate([[0.0], np.cumsum(phead)])  # prefix[s] = sum_{j<s}
    qtail = (q[:, _L - (_S - 1) :].astype(np.float64) ** 2).sum(axis=0)
    suffix = np.concatenate([[0.0], np.cumsum(qtail[::-1])])  # suffix[s] = last s

    losses = (SP - prefix[s] + SQ - suffix[s] - 2.0 * X) / (
        float(_B) * (_L - s).astype(np.float64)
    )
    return np.asarray(losses.min(), dtype=np.float32)


# revision 21
# speedup vs baseline: 17.6047x; 1.8014x over previous
"""OffsetMSE loss kernel for 8x Trainium2 NeuronCores.

Math: reference computes, for shifts s in 1..95,
    loss(s) = sum_b sum_{i<L-s} (p[b,i+s] - q[b,i])^2 / (B*(L-s))
and returns min_s loss(s).

Decomposition:
    loss(s)*B*(L-s) = A(s) - 2*X(s) + C(s)
      A(s) = SP - prefix_p(s)        SP = sum p^2,  prefix_p(s) = sum_b sum_{j<s} p^2
      C(s) = SQ - suffix_q(s)        SQ = sum q^2,  suffix_q(s) = sum_b sum_{i>=L-s} q^2
      X(s) = sum_b sum_j p[b,j+s]*q[b,j]   (p zero-padded past L)

The heavy terms (X for all 95 lags, SP, SQ) are computed on-device:
batch dim is sharded 2 sequences/core across 8 cores. Each core tiles its
data into "superblocks" of 128 rows x 2048, casts to fp16 (ACT casts p,
DVE casts q), and for each 128-column chunk runs PE matmuls with the q
chunk stationary:
  X:  moving = p window (128x224)  -> PSUM OUT[k,t]  = sum_u q[u+k]p[u+t]
  QQ: moving = q chunk  (128x128)  -> PSUM QQ[k,t']  = sum_u q[u+k]q[u+t']
  PP: stationary/moving = p chunk  -> PSUM PP[k,t']  = sum_u p[u+k]p[u+t']
accumulated over all chunks. Diagonal sums give X(s) = sum_k OUT[k,k+s],
SQ = tr(QQ), SP = tr(PP). Host combines partials (tiny, O(128*480)).
"""

import os
import sys

os.environ.setdefault("MYCRO_LOCAL_CACHE", "1")
if "/opt/trn_rl_repo" not in sys.path:
    sys.path.insert(0, "/opt/trn_rl_repo")

import numpy as np

_L = 1048576
_B = 16
_NCORES = 8
_BPC = _B // _NCORES  # sequences per core = 2
_P = 128
_W = 2048             # row width within a superblock
_SB = _W * _P         # superblock elements = 262144
_NSB = _L // _SB      # superblocks per sequence = 4
_NCHUNK = _W // 128   # 16
_S = 96               # max shift (exclusive); shifts used are 1..95
_NW = 128 + _S        # moving-operand window = 224
_PEXT = _W + _S       # p tile free extent = 2144 (rows overlap by 96)

TRACE = False
LAST_RESULTS = None

_NC_CACHE = None


def _build(rep=1, io_bufs=None, c16_bufs=None, loop_n=None):
    """rep>1 repeats the whole pass inside one NEFF (benchmarking only:
    output values then accumulate rep times). loop_n wraps the pass in a
    hardware For_i loop of loop_n iterations (benchmarking only)."""
    from concourse import bacc, mybir
    import concourse.bass as bass
    from concourse.tile import TileContext
    from contextlib import nullcontext

    if io_bufs is None:
        io_bufs = int(os.environ.get("K_IO_BUFS", "4"))
    if c16_bufs is None:
        c16_bufs = int(os.environ.get("K_C16_BUFS", "4"))
    split = int(os.environ.get("K_SPLIT", "2"))  # column-split units/superblock
    # ablation modes for benchmarking: full | xonly | nomm | dmaonly
    mode = os.environ.get("K_MODE", "full")

    f32 = mybir.dt.float32
    f16 = mybir.dt.float16

    nc = bacc.Bacc(
        "TRN2", target_bir_lowering=False, debug=False, enable_asserts=False
    )
    p_in = nc.dram_tensor("p", [_BPC, _L], f32, kind="ExternalInput")
    q_in = nc.dram_tensor("q", [_BPC, _L], f32, kind="ExternalInput")
    # xout columns: [0:224) X correlation matrix, [224:352) QQ, [352:480) PP
    _XC = _NW + 128 + 128
    xout = nc.dram_tensor("xout", [_P, _XC], f32, kind="ExternalOutput")

    nsb_total = _BPC * _NSB  # 8
    n_mms = rep * nsb_total * _NCHUNK  # 128 per PSUM target (per rep)

    with TileContext(nc) as tc:
        with (
            tc.tile_pool(name="io", bufs=io_bufs) as io_pool,
            tc.tile_pool(name="c16", bufs=c16_bufs) as c16_pool,
            tc.tile_pool(name="scr", bufs=1) as scr_pool,
            tc.tile_pool(name="psum", bufs=1, space="PSUM") as psum_pool,
        ):
            psum_x = psum_qq = psum_pp = None
            if mode in ("full", "xonly"):
                psum_x = psum_pool.tile([_P, _NW], f32, tag="px")
            if mode == "full":
                psum_qq = psum_pool.tile([_P, 128], f32, tag="pq")
                psum_pp = psum_pool.tile([_P, 128], f32, tag="pp")

            loop_ctx = tc.For_i(0, loop_n, 1) if loop_n else nullcontext()
            with loop_ctx:
                # Each superblock is processed in column units so casts and
                # matmuls pipeline behind the DMAs at sub-superblock
                # granularity (shrinks the serial tail after the last load).
                # `pattern` lists chunks per unit (sums to _NCHUNK); a small
                # final unit minimizes work after the very last input DMA.
                pat_env = os.environ.get("K_PATTERN", "")
                if pat_env:
                    pattern = [int(x) for x in pat_env.split(",")]
                else:
                    pattern = [_NCHUNK // split] * split
                assert sum(pattern) == _NCHUNK
                bounds = [0]
                for n in pattern:
                    bounds.append(bounds[-1] + n)
                mm = 0
                for b in range(rep * _BPC):
                    b = b % _BPC
                    for sb in range(_NSB):
                        off = b * _L + sb * _SB
                        last_sb = sb == _NSB - 1
                        p_tile = io_pool.tile([_P, _PEXT], f32, tag="p")
                        q_tile = io_pool.tile([_P, _W], f32, tag="q")
                        p16 = q16 = None
                        if mode != "dmaonly":
                            p16 = c16_pool.tile([_P, _PEXT], f16, tag="p16")
                            q16 = c16_pool.tile([_P, _W], f16, tag="q16")
                        if last_sb:
                            # row 127's tail would run past the sequence end ->
                            # zero it (p zero-padding). memset whole tail-col
                            # block (engines need aligned start partition);
                            # rows 0..126 are overwritten with real data below.
                            nc.vector.memset(p_tile[:, _W:_PEXT], 0.0)
                        for u in range(len(pattern)):
                            qlo, qhi = 128 * bounds[u], 128 * bounds[u + 1]
                            # p columns [plo, phi) for this unit (units chain
                            # non-overlapping; unit 0 includes the 96-col head)
                            plo = qlo + (_S if u > 0 else 0)
                            phi = qhi + _S
                            nc.sync.dma_start(
                                out=q_tile[:, qlo:qhi],
                                in_=bass.AP(
                                    q_in, off + qlo, [[_W, _P], [1, qhi - qlo]]
                                ),
                            )
                            if not (last_sb and u == len(pattern) - 1):
                                nc.sync.dma_start(
                                    out=p_tile[:, plo:phi],
                                    in_=bass.AP(
                                        p_in, off + plo, [[_W, _P], [1, phi - plo]]
                                    ),
                                )
                            else:
                                nc.sync.dma_start(
                                    out=p_tile[0 : _P - 1, plo:phi],
                                    in_=bass.AP(
                                        p_in,
                                        off + plo,
                                        [[_W, _P - 1], [1, phi - plo]],
                                    ),
                                )
                                nc.sync.dma_start(
                                    out=p_tile[_P - 1 : _P, plo:_W],
                                    in_=bass.AP(
                                        p_in,
                                        off + _W * (_P - 1) + plo,
                                        [[_W, 1], [1, _W - plo]],
                                    ),
                                )
                            if mode == "dmaonly":
                                mm += bounds[u + 1] - bounds[u]
                                continue
                            nc.scalar.copy(p16[:, plo:phi], p_tile[:, plo:phi])
                            nc.vector.tensor_copy(
                                q16[:, qlo:qhi], q_tile[:, qlo:qhi]
                            )
                            if mode == "nomm":
                                mm += bounds[u + 1] - bounds[u]
                                continue
                            for c in range(bounds[u], bounds[u + 1]):
                                first = mm == 0
                                last = mm == n_mms - 1
                                qc = q16[:, 128 * c : 128 * c + 128]
                                pc = p16[:, 128 * c : 128 * c + 128]
                                nc.tensor.matmul(
                                    psum_x[:, :],
                                    qc,
                                    p16[:, 128 * c : 128 * c + _NW],
                                    start=first,
                                    stop=last,
                                )
                                if mode == "full":
                                    nc.tensor.matmul(
                                        psum_qq[:, :], qc, qc,
                                        start=first, stop=last,
                                    )
                                    nc.tensor.matmul(
                                        psum_pp[:, :], pc, pc,
                                        start=first, stop=last,
                                    )
                                mm += 1

            out_sb = scr_pool.tile([_P, _XC], f32, tag="ox")
            if mode in ("dmaonly", "nomm", "xonly"):
                nc.vector.memset(out_sb[:, :], 0.0)
            if psum_x is not None:
                nc.vector.tensor_copy(out_sb[:, 0:_NW], psum_x[:, :])
            if psum_qq is not None:
                nc.scalar.copy(out_sb[:, _NW : _NW + 128], psum_qq[:, :])
                nc.vector.tensor_copy(out_sb[:, _NW + 128 : _XC], psum_pp[:, :])
            nc.sync.dma_start(out=xout[:, :], in_=out_sb[:, :])

    nc.compile()
    return nc


def _get_nc():
    global _NC_CACHE
    if _NC_CACHE is None:
        _NC_CACHE = _build()
    return _NC_CACHE


def _run_device(p, q):
    """p, q: (16, L) float32. Returns xout (128 x 480 f64) summed over cores."""
    global LAST_RESULTS
    from concourse import bass_utils

    nc = _get_nc()
    in_maps = [
        {
            "p": np.ascontiguousarray(p[_BPC * c : _BPC * (c + 1)]),
            "q": np.ascontiguousarray(q[_BPC * c : _BPC * (c + 1)]),
        }
        for c in range(_NCORES)
    ]
    if os.environ.get("BASS_BACKEND", "hw") == "sim":
        from concourse.bass_interp import CoreSim

        res_list = []
        for c in range(_NCORES):
            sim = CoreSim(nc)
            sim.tensor("p")[:] = in_maps[c]["p"]
            sim.tensor("q")[:] = in_maps[c]["q"]
            sim.simulate()
            res_list.append({"xout": np.array(sim.tensor("xout"))})
    else:
        res = bass_utils.run_bass_kernel_spmd(
            nc, in_maps, core_ids=list(range(_NCORES)), trace=TRACE
        )
        LAST_RESULTS = res
        res_list = res.results

    OUT = np.zeros((_P, _NW + 256), dtype=np.float64)
    for r in res_list:
        OUT += r["xout"].astype(np.float64)
    return OUT


def kernel(predict, target):
    p = np.ascontiguousarray(predict.reshape(_B, _L)).astype(np.float32, copy=False)
    q = np.ascontiguousarray(target.reshape(_B, _L)).astype(np.float32, copy=False)

    OUT = _run_device(p, q)

    s = np.arange(1, _S)  # shifts 1..95
    k = np.arange(_P)
    X = OUT[:, 0:_NW][k[:, None], k[:, None] + s[None, :]].sum(axis=0)  # (95,)
    SQ = np.trace(OUT[:, _NW : _NW + 128])
    SP = np.trace(OUT[:, _NW + 128 : _NW + 256])

    # tiny edge terms from the raw inputs (O(B*S) work)
    phead = (p[:, : _S - 1].astype(np.float64) ** 2).sum(axis=0)  # j = 0..94
    prefix = np.concatenate([[0.0], np.cumsum(phead)])  # prefix[s] = sum_{j<s}
    qtail = (q[:, _L - (_S - 1) :].astype(np.float64) ** 2).sum(axis=0)
    suffix = np.concatenate([[0.0], np.cumsum(qtail[::-1])])  # suffix[s] = last s

    losses = (SP - prefix[s] + SQ - suffix[s] - 2.0 * X) / (
        float(_B) * (_L - s).astype(np.float64)
    )
    return np.asarray(losses.min(), dtype=np.float32)
